# revision 70
# baseline (speedup 1.0000x reference)
"""Self-contained Trainium2 Bass kernel for nn_Attention_40226663694923.

Fused LayerNorm + multi-head attention + out-projection, sharded over
8 NeuronCores as (batch b in 0..3) x (head-group g in 0..1, 8 heads each).
Each core receives x[b].T plus its weight shards, computes a partial
out-projection [2048, 1024]; the host sums the two head-group partials
per batch and adds the bias.

v2 layout (post-trace rework of the 780us baseline):
  - Phases A (LN) and B (q/k/v projections) are fused per token-block so
    the PE never drains while the LN row chain (now reciprocal_approx_fast,
    not the 3.3us InstReciprocal) runs on DVE/ACT.
  - ALL q projections happen in phase B into a persistent q_sb, freeing
    PSUM in phase C (pl 4 banks + po 4 banks exactly fills PSUM).
  - Phase C inner loop alternates exp between the ACT engine (true Exp,
    bf16 out) and the DVE (Schraudolph bit-hack: one tensor_scalar
    f32->int16 whose bit pattern IS bf16 exp, ~3% max rel err) so neither
    engine gates the PE's logits+attnV stream.
  - Softmax tails use reciprocal_approx_fast + DRAM-bounce row broadcast
    on the gpsimd DMA queue; normalize multiplies on Pool/DVE.
All matmuls float32r or bf16 (both 1 cycle/row on the PE at N>=256).
"""

import os
import sys

for _p in ("/opt/trn_rl_repo", "/root/.axon_site/_ro/trn_rl_repo"):
    if os.path.isdir(_p) and _p not in sys.path:
        sys.path.append(_p)

from contextlib import ExitStack

import numpy as np

B, N, DIM = 4, 2048, 1024
H, D = 16, 64
HPC = 8        # heads per core
PAIRS = 4      # head pairs per core
KC = 8         # 1024 dim contraction chunks of 128
NB = 4         # token blocks of 512
TB = 512
TT = 16        # token tiles of 128
LN_EPS = 1e-6
N_CORES = 8

# Schraudolph bf16-exp constants: bits16(e^x) ~ trunc(x*A + Bc), viewed
# as bf16. Max rel err 3.3% over x in [-8, 6]; softmax-normalized error
# washes out to <0.2% in the attention output.
EXP_A = 184.6650558
EXP_B = 16251.0

_prog_cache = {}


def _build_program():
    import concourse.bass as bass
    import concourse.mybir as mybir
    import concourse.tile as tile
    from concourse import bacc
    from concourse.alu_op_type import AluOpType

    F32 = mybir.dt.float32
    F32R = mybir.dt.float32r
    BF16 = mybir.dt.bfloat16
    I16 = mybir.dt.int16
    AF = mybir.ActivationFunctionType

    dbg = bool(os.environ.get("ATTN_DEBUG_DUMP"))
    nc = bacc.Bacc("TRN2", target_bir_lowering=False, debug=False)
    xt_d = nc.dram_tensor("xt", [DIM, N], F32R, kind="ExternalInput")
    wq_d = nc.dram_tensor("wq", [128, KC, PAIRS, 128], F32R, kind="ExternalInput")
    wk_d = nc.dram_tensor("wk", [128, KC, PAIRS, 128], F32R, kind="ExternalInput")
    wv_d = nc.dram_tensor("wv", [128, KC, 512], F32R, kind="ExternalInput")
    wo_d = nc.dram_tensor("wo", [128, PAIRS, 1024], F32R, kind="ExternalInput")
    qb_d = nc.dram_tensor("qb", [PAIRS, 128], F32, kind="ExternalInput")
    kb_d = nc.dram_tensor("kb", [PAIRS, 128], F32, kind="ExternalInput")
    out_d = nc.dram_tensor("out", [N, DIM], F32, kind="ExternalOutput")
    if dbg:
        z_d = nc.dram_tensor("z_dbg", [DIM, N], F32, kind="ExternalOutput")
        k_dbg = nc.dram_tensor("k_dbg", [128, PAIRS, N], BF16, kind="ExternalOutput")
        q_dbg = nc.dram_tensor("q_dbg", [128, PAIRS, N], BF16, kind="ExternalOutput")
        v_dbg = nc.dram_tensor("v_dbg", [128, TT, HPC, D + 1], BF16,
                               kind="ExternalOutput")
        pl_dbg = nc.dram_tensor("pl_dbg", [2, 128, 1024], F32,
                                kind="ExternalOutput")
        ex_dbg = nc.dram_tensor("ex_dbg", [2, 128, 1024], BF16,
                                kind="ExternalOutput")
        den_dbg = nc.dram_tensor("den_dbg", [2, TB], F32, kind="ExternalOutput")
        ob_dbg = nc.dram_tensor("ob_dbg", [NB, 128, PAIRS, TB], F32R,
                                kind="ExternalOutput")

    with tile.TileContext(nc) as tc, ExitStack() as ctx:
        const_p = ctx.enter_context(tc.tile_pool(name="const", bufs=1))
        big_p = ctx.enter_context(tc.tile_pool(name="big", bufs=1))

        onesF = const_p.tile([128, 128], F32)
        nc.vector.memset(onesF, 1.0)
        ones_col = const_p.tile([128, 1], F32R)
        nc.vector.tensor_copy(out=ones_col, in_=onesF[:, 0:1])
        ones_row = const_p.tile([1, 128], F32R)
        nc.vector.tensor_copy(out=ones_row, in_=onesF[0:1, :])
        eps1 = const_p.tile([1, 1], F32)
        nc.vector.memset(eps1, LN_EPS)
        zb128 = const_p.tile([128, 1], F32)
        nc.vector.memset(zb128, 0.0)
        qb_sb = const_p.tile([128, PAIRS], F32)
        kb_sb = const_p.tile([128, PAIRS], F32)
        for pair in range(PAIRS):
            nc.gpsimd.dma_start(
                out=qb_sb[:, pair : pair + 1],
                in_=qb_d[pair, :].rearrange("(p one) -> p one", one=1),
            )
            nc.gpsimd.dma_start(
                out=kb_sb[:, pair : pair + 1],
                in_=kb_d[pair, :].rearrange("(p one) -> p one", one=1),
            )

        # persistent big tensors
        xt_sb = big_p.tile([128, KC, N], F32R)      # becomes z (normalized) in place
        k_sb = big_p.tile([128, PAIRS, N], BF16)    # kT, two heads packed per pair
        q_sb = big_p.tile([128, PAIRS, N], BF16)    # qT, same packing as k
        v_sb = big_p.tile([128, TT, HPC, D + 1], BF16)  # V natural + ones column
        wo_sb = big_p.tile([128, PAIRS, 1024], F32R)

        # xt: per-(kc, tb) pieces so phase A's stats for tb=0 can start after
        # only 8 small DMAs (tb-major issue order). sync queue.
        def xt_dma(q, tb):
            for kc in range(KC):
                q.dma_start(
                    out=xt_sb[:, kc, tb * TB : (tb + 1) * TB],
                    in_=xt_d[kc * 128 : (kc + 1) * 128, tb * TB : (tb + 1) * TB],
                )

        nc.vector.tensor_copy(
            out=v_sb[:, :, :, D : D + 1],
            in_=onesF.rearrange("p (a b c) -> p a b c", a=TT, b=HPC),
        )
        psB = ctx.enter_context(tc.tile_pool(name="psB", bufs=2, space="PSUM"))
        # DMAs split across the sync and gpsimd queues, ordered by first use:
        # stats(tb0/tb1) -> k proj (wk) -> v proj (wv) -> q proj (wq) ->
        # stats(tb2/tb3) -> out proj (wo).
        # wq lives in its own ctx-managed pool: the deferred q-proj of tb3
        # reads it inside phase C, after the wkv pool has closed.
        wq_pool = ctx.enter_context(tc.tile_pool(name="wqp", bufs=1))
        wq_sb = wq_pool.tile([128, KC, PAIRS, 128], F32R)
        wkv_ctx = tc.tile_pool(name="wkv", bufs=1)
        wkv_p = wkv_ctx.__enter__()
        wv_sb = wkv_p.tile([128, KC, 512], F32R, tag="wv")
        wk_sb = wkv_p.tile([128, KC, PAIRS, 128], F32R, tag="wk")
        xt_dma(nc.sync, 0)
        xt_dma(nc.gpsimd, 1)
        nc.sync.dma_start(out=wk_sb, in_=wk_d[:, :, :, :])
        nc.gpsimd.dma_start(out=wv_sb, in_=wv_d[:, :, :])
        xt_dma(nc.sync, 2)
        xt_dma(nc.sync, 3)
        nc.sync.dma_start(out=wo_sb, in_=wo_d[:, :, :])

        # ------------- Phase A+B fused: LN and q/k/v projections ------------
        psA_ctx = tc.tile_pool(name="psA", bufs=1, space="PSUM")
        psA = psA_ctx.__enter__()
        sq_ctx = tc.tile_pool(name="sqp", bufs=3)
        sqp = sq_ctx.__enter__()
        rows_ctx = tc.tile_pool(name="rows", bufs=1)
        rows = rows_ctx.__enter__()

        def emit_stats(tb):
            ts_ = slice(tb * TB, (tb + 1) * TB)
            s1 = psA.tile([1, TB], F32, tag="s1", bufs=1)
            s2 = psA.tile([1, TB], F32, tag="s2", bufs=1)
            for kc in range(KC):
                sq = sqp.tile([128, TB], F32R, tag="sq")
                nc.scalar.activation(out=sq, in_=xt_sb[:, kc, ts_].bitcast(F32),
                                     func=AF.Square, bias=zb128[:, 0:1])
                nc.tensor.matmul(s1, lhsT=ones_col, rhs=xt_sb[:, kc, ts_],
                                 start=(kc == 0), stop=(kc == KC - 1))
                nc.tensor.matmul(s2, lhsT=ones_col, rhs=sq,
                                 start=(kc == 0), stop=(kc == KC - 1))
            return s1, s2

        def emit_rows(tb, s1, s2):
            mu = rows.tile([1, TB], F32, tag="mu")
            nc.scalar.mul(mu, s1, 1.0 / DIM)
            ex2 = rows.tile([1, TB], F32, tag="ex2")
            nc.scalar.mul(ex2, s2, 1.0 / DIM)
            var_r = rows.tile([1, TB], F32, tag="var")
            nc.vector.tensor_mul(var_r, mu, mu)
            nc.vector.tensor_sub(var_r, ex2, var_r)
            sd = rows.tile([1, TB], F32, tag="sd")
            nc.scalar.activation(out=sd, in_=var_r, func=AF.Sqrt,
                                 bias=eps1[0:1, 0:1])
            rstd_r = rows.tile([1, TB], F32, tag="rstd_r")
            nc.vector.reciprocal_approx_fast(out=rstd_r, in_=sd)
            murstd_r = rows.tile([1, TB], F32R, tag="murstd")
            nc.vector.tensor_mul(murstd_r, mu, rstd_r)
            rstd_rr = rows.tile([1, TB], F32R, tag="rstd_rr")
            nc.vector.tensor_copy(out=rstd_rr, in_=rstd_r)
            return rstd_rr, murstd_r

        def emit_rb(tb, rstd_rr, murstd_r):
            rb1 = psA.tile([128, TB], F32, tag="rb1", bufs=1)
            nc.tensor.matmul(rb1, lhsT=ones_row, rhs=rstd_rr,
                             start=True, stop=True)
            rb2 = psA.tile([128, TB], F32, tag="rb2", bufs=1)
            nc.tensor.matmul(rb2, lhsT=ones_row, rhs=murstd_r,
                             start=True, stop=True)
            return rb1, rb2

        def emit_norm(tb, rb1, rb2):
            ts_ = slice(tb * TB, (tb + 1) * TB)
            for kc in range(KC):
                nc.vector.tensor_mul(xt_sb[:, kc, ts_],
                                     xt_sb[:, kc, ts_].bitcast(F32), rb1)
                nc.vector.tensor_sub(xt_sb[:, kc, ts_],
                                     xt_sb[:, kc, ts_].bitcast(F32), rb2)

        def emit_qproj(tb):
            ts_ = slice(tb * TB, (tb + 1) * TB)
            for pair in range(PAIRS):
                pq = psB.tile([128, TB], F32, tag="po", bufs=4)
                for kc in range(KC):
                    nc.tensor.matmul(pq, lhsT=wq_sb[:, kc, pair, :],
                                     rhs=xt_sb[:, kc, ts_],
                                     start=(kc == 0), stop=(kc == KC - 1))
                nc.scalar.activation(out=q_sb[:, pair, ts_], in_=pq,
                                     func=AF.Identity,
                                     bias=qb_sb[:, pair : pair + 1])

        def emit_proj(tb, include_q=True):
            ts_ = slice(tb * TB, (tb + 1) * TB)
            # kc-major across all 4 pairs (4 live PSUM accumulators): the PE
            # consumes each just-normalized kc tile for every pair at once
            # instead of head-of-line blocking on pair 0's later kc tiles
            # while the DVE normalize is still producing them.
            pks = {}
            for pair in range(PAIRS):
                pk = psB.tile([128, TB], F32, tag="po", bufs=4)
                pks[pair] = pk
            for kc in range(KC):
                for pair in range(PAIRS):
                    nc.tensor.matmul(pks[pair], lhsT=wk_sb[:, kc, pair, :],
                                     rhs=xt_sb[:, kc, ts_],
                                     start=(kc == 0), stop=(kc == KC - 1))
            for pair in range(PAIRS):
                # k bias-add on ACT (Identity w/ per-partition bias), bf16 out
                nc.scalar.activation(out=k_sb[:, pair, ts_], in_=pks[pair],
                                     func=AF.Identity,
                                     bias=kb_sb[:, pair : pair + 1])
            for tt in range(tb * 4, tb * 4 + 4):
                tts = slice(tt * 128, (tt + 1) * 128)
                pv = psB.tile([128, 512], F32, tag="po", bufs=4)
                for kc in range(KC):
                    nc.tensor.matmul(pv, lhsT=xt_sb[:, kc, tts],
                                     rhs=wv_sb[:, kc, :],
                                     start=(kc == 0), stop=(kc == KC - 1))
                nc.scalar.copy(
                    out=v_sb[:, tt, :, 0:D],
                    in_=pv.rearrange("p (h d) -> p h d", h=HPC),
                )
            if include_q:
                emit_qproj(tb)

        # interleaved emission: PE stream = stats0 rb0 stats1 proj0 rb1
        # stats2 proj1 rb2 stats3 proj2 rb3 proj3 — row chains and
        # normalizes hide under the previous tb's projections.
        s_t = {}
        r_t = {}
        s_t[0] = emit_stats(0)
        r_t[0] = emit_rows(0, *s_t[0])
        rb = emit_rb(0, *r_t[0])
        emit_norm(0, *rb)
        s_t[1] = emit_stats(1)
        r_t[1] = emit_rows(1, *s_t[1])
        # wq on the ACT queue, after tb0/tb1 Squares: arrives just before
        # proj(0)'s q projections (which run last within the block).
        nc.scalar.dma_start(out=wq_sb, in_=wq_d[:, :, :, :])
        emit_proj(0)
        for tb in range(1, NB):
            rb = emit_rb(tb, *r_t[tb])
            emit_norm(tb, *rb)
            if tb + 1 < NB:
                s_t[tb + 1] = emit_stats(tb + 1)
                r_t[tb + 1] = emit_rows(tb + 1, *s_t[tb + 1])
            # tb3's q projection is deferred into phase C's ramp-up: C only
            # needs k/v of tb3 to start, and q[tqb3] isn't read until the
            # last quarter of phase C.
            emit_proj(tb, include_q=(tb != NB - 1))

        if dbg:
            for kc in range(KC):
                nc.sync.dma_start(
                    out=z_d[kc * 128 : (kc + 1) * 128, :],
                    in_=xt_sb[:, kc, :].bitcast(F32))
            nc.sync.dma_start(out=k_dbg[:, :, :], in_=k_sb[:, :, :])
            nc.sync.dma_start(out=q_dbg[:, :, :], in_=q_sb[:, :, :])
            nc.sync.dma_start(out=v_dbg[:, :, :, :], in_=v_sb[:, :, :, :])

        rows_ctx.__exit__(None, None, None)
        sq_ctx.__exit__(None, None, None)
        psA_ctx.__exit__(None, None, None)
        wkv_ctx.__exit__(None, None, None)

        # ---------------- Phase C: attention + out-projection -----------------
        psC = ctx.enter_context(tc.tile_pool(name="psC", bufs=1, space="PSUM"))
        with tc.tile_pool(name="attn", bufs=2) as ap_, \
             tc.tile_pool(name="rows2", bufs=2) as rows2, \
             tc.tile_pool(name="drbounce", bufs=4, space="DRAM") as dram_p:
            pending_out = [None]
            pending_t = [None]
            for tqb in range(NB):
                tqs_ = slice(tqb * TB, (tqb + 1) * TB)
                obuf = ap_.tile([128, PAIRS, TB], F32R, tag="ob", bufs=2)

                def emit_tail(tpair, hh, po, tobuf):
                    # Softmax-normalize tail (one head) with NO PE work. One
                    # ACT copy evicts the whole [o|den] tile to SBUF so the
                    # PSUM bank is free for the next pair-group immediately
                    # (the DRAM-bounce round trip then happens off the po
                    # reuse path): fast-approx reciprocal of the den row,
                    # row-broadcast via DRAM-bounce DMA (gpsimd), DVE mult.
                    import concourse.bass as _b
                    drow = rows2.tile([1, TB], F32, tag="dn")
                    nc.scalar.copy(out=drow, in_=po[D : D + 1, :])
                    ot = ap_.tile([D, TB], F32, tag="ot", bufs=4)
                    nc.scalar.copy(out=ot, in_=po[0:D, :])
                    rrow = rows2.tile([1, TB], F32, tag="rr")
                    # custom-DVE ops need SBUF inputs at partition 0 on HW
                    nc.vector.reciprocal_approx_fast(out=rrow, in_=drow)
                    dr = dram_p.tile([1, TB], F32, tag="dr")
                    # write and broadcast-read on DIFFERENT queues: forces an
                    # explicit completion semaphore between them (same-queue
                    # DMA descriptors can execute on parallel channels, which
                    # intermittently raced the read against the write).
                    nc.sync.dma_start(out=dr, in_=rrow)
                    rb_ = ap_.tile([64, TB], F32, tag="rb", bufs=2)
                    bc = _b.AP(tensor=dr.tensor, offset=dr.offset,
                               ap=[[0, 64]] + [list(p) for p in dr[0, :].ap])
                    nc.gpsimd.dma_start(out=rb_, in_=bc)
                    if hh == 0:
                        nc.gpsimd.tensor_mul(tobuf[0:64, tpair, :],
                                             ot[0:D, :], rb_)
                    else:
                        tmp = ap_.tile([64, TB], F32R, tag="tmp")
                        nc.gpsimd.tensor_mul(tmp, ot[0:D, :], rb_)
                        nc.gpsimd.dma_start(out=tobuf[64:128, tpair, :],
                                            in_=tmp)

                def emit_av(vpair, vtkc, vex, vpo2):
                    # attn@V for k-tile vtkc (one pair), one step late so the
                    # PE never queues behind an in-flight exp.
                    for hh in range(2):
                        nc.tensor.matmul(
                            vpo2[hh][0 : D + 1, :],
                            lhsT=v_sb[:, vtkc, vpair * 2 + hh, :],
                            rhs=vex[:, hh * 512 : (hh + 1) * 512],
                            start=(vtkc == 0), stop=(vtkc == 2 * KC - 1))

                # Two pairs stream concurrently: pair A's exp (ACT) hides
                # under pair B's logits and attn@V, pair B's exp (DVE/
                # Schraudolph) under pair A's. Per-pair pl is [128,1024]
                # (1-tkc deep): lg(t+1) waits exp(t), which completes during
                # the other pair's 4 PE matmuls. pl 2x2 banks + po 4 = PSUM.
                for pg in range(PAIRS // 2):
                    prs = (2 * pg, 2 * pg + 1)
                    pos = {}
                    pls = {}
                    for sl, pair in enumerate(prs):
                        po0 = psB.tile([128, TB], F32, tag="po", bufs=4)
                        po1 = psB.tile([128, TB], F32, tag="po", bufs=4)
                        pos[pair] = [po0, po1]
                        pl_t = psC.tile([128, 1024], F32, tag="pl", bufs=2)
                        pls[pair] = pl_t
                    pend = []
                    for tkc in range(2 * KC):
                        pend_new = []
                        for sl, pair in enumerate(prs):
                            pl = pls[pair]
                            ex = ap_.tile([128, 1024], BF16, tag="ex", bufs=6)
                            for hh in range(2):
                                pb = hh * 64
                                nc.tensor.matmul(
                                    pl[:, hh * 512 : (hh + 1) * 512],
                                    lhsT=k_sb[pb : pb + 64, pair,
                                              tkc * 128 : (tkc + 1) * 128],
                                    rhs=q_sb[pb : pb + 64, pair, tqs_],
                                    start=True, stop=True)
                            if tkc == 2 * KC - 1:
                                # last tkc of the group: split the exp across
                                # BOTH engines (one head-half each) so the pl
                                # banks free ~half an exp earlier -- the next
                                # group's first logits wait on exactly this.
                                h_act = 0 if sl == 0 else 1
                                ha, hd = h_act * 512, (1 - h_act) * 512
                                nc.scalar.activation(
                                    out=ex[:, ha : ha + 512],
                                    in_=pl[:, ha : ha + 512],
                                    func=AF.Exp, bias=zb128[:, 0:1])
                                nc.vector.tensor_scalar(
                                    out=ex.bitcast(I16)[:, hd : hd + 512],
                                    in0=pl[:, hd : hd + 512],
                                    scalar1=EXP_A, scalar2=EXP_B,
                                    op0=AluOpType.mult, op1=AluOpType.add)
                            elif sl == 0:
                                nc.scalar.activation(
                                    out=ex, in_=pl[:, :],
                                    func=AF.Exp, bias=zb128[:, 0:1])
                            else:
                                # Schraudolph: bf16-exp bit pattern via one
                                # DVE tensor_scalar (f32->int16 trunc).
                                nc.vector.tensor_scalar(
                                    out=ex.bitcast(I16), in0=pl[:, :],
                                    scalar1=EXP_A, scalar2=EXP_B,
                                    op0=AluOpType.mult, op1=AluOpType.add)
                            pend_new.append((pair, tkc, ex, pos[pair]))
                        for args in pend:
                            emit_av(*args)
                        pend = pend_new
                        # spread the previous group's 4 tail chains across
                        # tkc 1..4 so the exp pipeline isn't disrupted by a
                        # burst of ACT/DVE tail work at the group boundary
                        # (carried across tqb boundaries too).
                        if pending_t[0] and 1 <= tkc <= len(pending_t[0]):
                            emit_tail(*pending_t[0][tkc - 1])
                            if tkc == len(pending_t[0]):
                                pending_t[0] = None
                        if tqb == 0 and pg == 0 and tkc == 2:
                            # deferred tb3 q-projection fills the PE while
                            # the exp pipeline is still ramping up.
                            emit_qproj(NB - 1)
                    for args in pend:
                        emit_av(*args)
                    pending_t[0] = [(pair, hh, pos[pair][hh], obuf)
                                    for pair in prs for hh in range(2)]
                    if pg == 0 and pending_out[0] is not None:
                        pending_out[0]()
                        pending_out[0] = None
                if dbg:
                    nc.sync.dma_start(out=ob_dbg[tqb, :, :, :],
                                      in_=obuf[:, :, :])

                def make_outproj(otqb, oobuf):
                    def emit():
                        for tqs in range(4):
                            osl = slice(tqs * 128, (tqs + 1) * 128)
                            osb = ap_.tile([128, 1024], F32, tag="osb", bufs=2)
                            pc0 = psB.tile([128, TB], F32, tag="po", bufs=4)
                            pc1 = psB.tile([128, TB], F32, tag="po", bufs=4)
                            for nh, pc in ((0, pc0), (1, pc1)):
                                for j in range(PAIRS):
                                    nc.tensor.matmul(
                                        pc, lhsT=oobuf[:, j, osl],
                                        rhs=wo_sb[:, j, nh * 512 : (nh + 1) * 512],
                                        start=(j == 0), stop=(j == PAIRS - 1))
                                if nh == 0:
                                    nc.scalar.copy(out=osb[:, 0:512], in_=pc)
                                else:
                                    nc.vector.tensor_copy(out=osb[:, 512:1024],
                                                          in_=pc)
                            r0 = otqb * TB + tqs * 128
                            nc.sync.dma_start(out=out_d[r0 : r0 + 128, :],
                                              in_=osb)
                    return emit

                if tqb == NB - 1:
                    for tp in pending_t[0]:
                        emit_tail(*tp)
                    pending_t[0] = None
                    make_outproj(tqb, obuf)()
                else:
                    pending_out[0] = make_outproj(tqb, obuf)
    nc.finalize()
    return nc


def get_program():
    if "nc" not in _prog_cache:
        _prog_cache["nc"] = _build_program()
    return _prog_cache["nc"]


def _round_f32r(a):
    """Round fp32 to fp32r (E8M11: 11 mantissa bits, low 12 bits zero),
    round-to-nearest-even. Matches the PE's fp32r operand precision so the
    DMA-loaded tensors satisfy walrus's 'rounded to FP32r' requirement."""
    b = np.ascontiguousarray(a, np.float32).view(np.uint32)
    lsb = (b >> np.uint32(12)) & np.uint32(1)
    r = (b + np.uint32(0x7FF) + lsb) & np.uint32(0xFFFFF000)
    return r.view(np.float32)


def _pack_inputs(x, ln_scale, ln_bias, w_qkv, w_out, b_out):
    """Returns (in_maps for 8 cores, per-batch host bias [1024])."""
    x = np.ascontiguousarray(np.asarray(x, np.float32))
    ln_scale = np.asarray(ln_scale, np.float32)
    ln_bias = np.asarray(ln_bias, np.float32)
    w_qkv = np.asarray(w_qkv, np.float32)
    w_out = np.asarray(w_out, np.float32)
    b_out = np.asarray(b_out, np.float32)

    ws = w_qkv * ln_scale[:, None]          # fold LN scale into weights
    wq_all = ws[:, 0:1024] * (D ** -0.5)    # fold 1/sqrt(d) into q
    wk_all = ws[:, 1024:2048]
    wv_all = ws[:, 2048:3072]
    qb_all = (ln_bias @ w_qkv[:, 0:1024]) * (D ** -0.5)
    kb_all = ln_bias @ w_qkv[:, 1024:2048]
    vb_all = ln_bias @ w_qkv[:, 2048:3072]
    b_eff = (b_out + vb_all @ w_out).astype(np.float32)  # host-side bias

    in_maps = []
    for core in range(N_CORES):
        b_i, g = core // 2, core % 2
        cs = slice(g * 512, (g + 1) * 512)
        # [dim, 8 heads, 64] -> pairs of heads packed along m
        wq_g = wq_all[:, cs].reshape(DIM, PAIRS, 128)   # [dim, pair, 2*64]
        wk_g = wk_all[:, cs].reshape(DIM, PAIRS, 128)
        # -> [p, kc, pair, m] so one whole-tensor DMA is contiguous
        wq_p = np.ascontiguousarray(
            wq_g.reshape(KC, 128, PAIRS, 128).transpose(1, 0, 2, 3))
        wk_p = np.ascontiguousarray(
            wk_g.reshape(KC, 128, PAIRS, 128).transpose(1, 0, 2, 3))
        wv_p = np.ascontiguousarray(
            wv_all[:, cs].reshape(KC, 128, 512).transpose(1, 0, 2))
        wo_p = np.ascontiguousarray(
            w_out[cs, :].reshape(PAIRS, 128, DIM).transpose(1, 0, 2))
        qb_p = np.ascontiguousarray(qb_all[cs].reshape(PAIRS, 128))
        kb_p = np.ascontiguousarray(kb_all[cs].reshape(PAIRS, 128))
        xt = np.ascontiguousarray(x[b_i].T)
        in_maps.append({
            "xt": _round_f32r(xt), "wq": _round_f32r(wq_p),
            "wk": _round_f32r(wk_p), "wv": _round_f32r(wv_p),
            "wo": _round_f32r(wo_p), "qb": qb_p, "kb": kb_p,
        })
    return in_maps, b_eff


def kernel(x, ln_scale, ln_bias, w_qkv, w_out, b_out):
    from concourse.bass_utils import run_bass_kernel_spmd

    nc = get_program()
    in_maps, b_eff = _pack_inputs(x, ln_scale, ln_bias, w_qkv, w_out, b_out)
    trace = bool(os.environ.get("ATTN_KERNEL_TRACE"))
    res = run_bass_kernel_spmd(nc, in_maps, core_ids=list(range(N_CORES)),
                               trace=trace)
    _prog_cache["last_exec_time_ns"] = res.exec_time_ns
    _prog_cache["last_result"] = res
    outs = res.results
    out = np.empty((B, N, DIM), np.float32)
    for b in range(B):
        out[b] = outs[2 * b]["out"] + outs[2 * b + 1]["out"] + b_eff
    return out


# revision 71
# speedup vs baseline: 1.0106x; 1.0106x over previous
"""Self-contained Trainium2 Bass kernel for nn_Attention_40226663694923.

Fused LayerNorm + multi-head attention + out-projection, sharded over
8 NeuronCores as (batch b in 0..3) x (head-group g in 0..1, 8 heads each).
Each core receives x[b].T plus its weight shards, computes a partial
out-projection [2048, 1024]; the host sums the two head-group partials
per batch and adds the bias.

v2 layout (post-trace rework of the 780us baseline):
  - Phases A (LN) and B (q/k/v projections) are fused per token-block so
    the PE never drains while the LN row chain (now reciprocal_approx_fast,
    not the 3.3us InstReciprocal) runs on DVE/ACT.
  - ALL q projections happen in phase B into a persistent q_sb, freeing
    PSUM in phase C (pl 4 banks + po 4 banks exactly fills PSUM).
  - Phase C inner loop alternates exp between the ACT engine (true Exp,
    bf16 out) and the DVE (Schraudolph bit-hack: one tensor_scalar
    f32->int16 whose bit pattern IS bf16 exp, ~3% max rel err) so neither
    engine gates the PE's logits+attnV stream.
  - Softmax tails use reciprocal_approx_fast + DRAM-bounce row broadcast
    on the gpsimd DMA queue; normalize multiplies on Pool/DVE.
All matmuls float32r or bf16 (both 1 cycle/row on the PE at N>=256).
"""

import os
import sys

for _p in ("/opt/trn_rl_repo", "/root/.axon_site/_ro/trn_rl_repo"):
    if os.path.isdir(_p) and _p not in sys.path:
        sys.path.append(_p)

from contextlib import ExitStack

import numpy as np

B, N, DIM = 4, 2048, 1024
H, D = 16, 64
HPC = 8        # heads per core
PAIRS = 4      # head pairs per core
KC = 8         # 1024 dim contraction chunks of 128
NB = 4         # token blocks of 512
TB = 512
TT = 16        # token tiles of 128
LN_EPS = 1e-6
N_CORES = 8

# Schraudolph bf16-exp constants: bits16(e^x) ~ trunc(x*A + Bc), viewed
# as bf16. Max rel err 3.3% over x in [-8, 6]; softmax-normalized error
# washes out to <0.2% in the attention output.
EXP_A = 184.6650558
EXP_B = 16251.0

_prog_cache = {}


def _build_program():
    import concourse.bass as bass
    import concourse.mybir as mybir
    import concourse.tile as tile
    from concourse import bacc
    from concourse.alu_op_type import AluOpType

    F32 = mybir.dt.float32
    F32R = mybir.dt.float32r
    BF16 = mybir.dt.bfloat16
    I16 = mybir.dt.int16
    AF = mybir.ActivationFunctionType

    dbg = bool(os.environ.get("ATTN_DEBUG_DUMP"))
    nc = bacc.Bacc("TRN2", target_bir_lowering=False, debug=False)
    xt_d = nc.dram_tensor("xt", [DIM, N], F32R, kind="ExternalInput")
    wq_d = nc.dram_tensor("wq", [128, KC, PAIRS, 128], F32R, kind="ExternalInput")
    wk_d = nc.dram_tensor("wk", [128, KC, PAIRS, 128], F32R, kind="ExternalInput")
    wv_d = nc.dram_tensor("wv", [128, KC, 512], F32R, kind="ExternalInput")
    wo_d = nc.dram_tensor("wo", [128, PAIRS, 1024], F32R, kind="ExternalInput")
    qb_d = nc.dram_tensor("qb", [PAIRS, 128], F32, kind="ExternalInput")
    kb_d = nc.dram_tensor("kb", [PAIRS, 128], F32, kind="ExternalInput")
    out_d = nc.dram_tensor("out", [N, DIM], F32, kind="ExternalOutput")
    if dbg:
        z_d = nc.dram_tensor("z_dbg", [DIM, N], F32, kind="ExternalOutput")
        k_dbg = nc.dram_tensor("k_dbg", [128, PAIRS, N], BF16, kind="ExternalOutput")
        q_dbg = nc.dram_tensor("q_dbg", [128, PAIRS, N], BF16, kind="ExternalOutput")
        v_dbg = nc.dram_tensor("v_dbg", [128, TT, HPC, D + 1], BF16,
                               kind="ExternalOutput")
        pl_dbg = nc.dram_tensor("pl_dbg", [2, 128, 1024], F32,
                                kind="ExternalOutput")
        ex_dbg = nc.dram_tensor("ex_dbg", [2, 128, 1024], BF16,
                                kind="ExternalOutput")
        den_dbg = nc.dram_tensor("den_dbg", [2, TB], F32, kind="ExternalOutput")
        ob_dbg = nc.dram_tensor("ob_dbg", [NB, 128, PAIRS, TB], F32R,
                                kind="ExternalOutput")

    with tile.TileContext(nc) as tc, ExitStack() as ctx:
        const_p = ctx.enter_context(tc.tile_pool(name="const", bufs=1))
        big_p = ctx.enter_context(tc.tile_pool(name="big", bufs=1))

        onesF = const_p.tile([128, 128], F32)
        nc.vector.memset(onesF, 1.0)
        ones_col = const_p.tile([128, 1], F32R)
        nc.vector.tensor_copy(out=ones_col, in_=onesF[:, 0:1])
        ones_row = const_p.tile([1, 128], F32R)
        nc.vector.tensor_copy(out=ones_row, in_=onesF[0:1, :])
        eps1 = const_p.tile([1, 1], F32)
        nc.vector.memset(eps1, LN_EPS)
        zb128 = const_p.tile([128, 1], F32)
        nc.vector.memset(zb128, 0.0)
        qb_sb = const_p.tile([128, PAIRS], F32)
        kb_sb = const_p.tile([128, PAIRS], F32)
        for pair in range(PAIRS):
            nc.gpsimd.dma_start(
                out=qb_sb[:, pair : pair + 1],
                in_=qb_d[pair, :].rearrange("(p one) -> p one", one=1),
            )
            nc.gpsimd.dma_start(
                out=kb_sb[:, pair : pair + 1],
                in_=kb_d[pair, :].rearrange("(p one) -> p one", one=1),
            )

        # persistent big tensors
        xt_sb = big_p.tile([128, KC, N], F32R)      # becomes z (normalized) in place
        k_sb = big_p.tile([128, PAIRS, N], BF16)    # kT, two heads packed per pair
        q_sb = big_p.tile([128, PAIRS, N], BF16)    # qT, same packing as k
        v_sb = big_p.tile([128, TT, HPC, D + 1], BF16)  # V natural + ones column
        wo_sb = big_p.tile([128, PAIRS, 1024], F32R)

        # xt: per-(kc, tb) pieces so phase A's stats for tb=0 can start after
        # only 8 small DMAs (tb-major issue order). sync queue.
        def xt_dma(q, tb):
            for kc in range(KC):
                q.dma_start(
                    out=xt_sb[:, kc, tb * TB : (tb + 1) * TB],
                    in_=xt_d[kc * 128 : (kc + 1) * 128, tb * TB : (tb + 1) * TB],
                )

        nc.vector.tensor_copy(
            out=v_sb[:, :, :, D : D + 1],
            in_=onesF.rearrange("p (a b c) -> p a b c", a=TT, b=HPC),
        )
        psB = ctx.enter_context(tc.tile_pool(name="psB", bufs=2, space="PSUM"))
        # DMAs split across the sync and gpsimd queues, ordered by first use:
        # stats(tb0/tb1) -> k proj (wk) -> v proj (wv) -> q proj (wq) ->
        # stats(tb2/tb3) -> out proj (wo).
        # wq lives in its own ctx-managed pool: the deferred q-proj of tb3
        # reads it inside phase C, after the wkv pool has closed.
        wq_pool = ctx.enter_context(tc.tile_pool(name="wqp", bufs=1))
        wq_sb = wq_pool.tile([128, KC, PAIRS, 128], F32R)
        wkv_ctx = tc.tile_pool(name="wkv", bufs=1)
        wkv_p = wkv_ctx.__enter__()
        wv_sb = wkv_p.tile([128, KC, 512], F32R, tag="wv")
        wk_sb = wkv_p.tile([128, KC, PAIRS, 128], F32R, tag="wk")
        xt_dma(nc.sync, 0)
        xt_dma(nc.gpsimd, 1)
        nc.sync.dma_start(out=wk_sb, in_=wk_d[:, :, :, :])
        nc.gpsimd.dma_start(out=wv_sb, in_=wv_d[:, :, :])
        xt_dma(nc.sync, 2)
        xt_dma(nc.sync, 3)
        nc.sync.dma_start(out=wo_sb, in_=wo_d[:, :, :])

        # ------------- Phase A+B fused: LN and q/k/v projections ------------
        psA_ctx = tc.tile_pool(name="psA", bufs=1, space="PSUM")
        psA = psA_ctx.__enter__()
        sq_ctx = tc.tile_pool(name="sqp", bufs=3)
        sqp = sq_ctx.__enter__()
        rows_ctx = tc.tile_pool(name="rows", bufs=1)
        rows = rows_ctx.__enter__()

        def emit_stats(tb):
            ts_ = slice(tb * TB, (tb + 1) * TB)
            s1 = psA.tile([1, TB], F32, tag="s1", bufs=1)
            s2 = psA.tile([1, TB], F32, tag="s2", bufs=1)
            for kc in range(KC):
                sq = sqp.tile([128, TB], F32R, tag="sq")
                nc.scalar.activation(out=sq, in_=xt_sb[:, kc, ts_].bitcast(F32),
                                     func=AF.Square, bias=zb128[:, 0:1])
                nc.tensor.matmul(s1, lhsT=ones_col, rhs=xt_sb[:, kc, ts_],
                                 start=(kc == 0), stop=(kc == KC - 1))
                nc.tensor.matmul(s2, lhsT=ones_col, rhs=sq,
                                 start=(kc == 0), stop=(kc == KC - 1))
            return s1, s2

        def emit_rows(tb, s1, s2):
            mu = rows.tile([1, TB], F32, tag="mu")
            nc.scalar.mul(mu, s1, 1.0 / DIM)
            ex2 = rows.tile([1, TB], F32, tag="ex2")
            nc.scalar.mul(ex2, s2, 1.0 / DIM)
            var_r = rows.tile([1, TB], F32, tag="var")
            nc.vector.tensor_mul(var_r, mu, mu)
            nc.vector.tensor_sub(var_r, ex2, var_r)
            sd = rows.tile([1, TB], F32, tag="sd")
            nc.scalar.activation(out=sd, in_=var_r, func=AF.Sqrt,
                                 bias=eps1[0:1, 0:1])
            rstd_r = rows.tile([1, TB], F32, tag="rstd_r")
            nc.vector.reciprocal_approx_fast(out=rstd_r, in_=sd)
            murstd_r = rows.tile([1, TB], F32R, tag="murstd")
            nc.vector.tensor_mul(murstd_r, mu, rstd_r)
            rstd_rr = rows.tile([1, TB], F32R, tag="rstd_rr")
            nc.vector.tensor_copy(out=rstd_rr, in_=rstd_r)
            return rstd_rr, murstd_r

        def emit_rb(tb, rstd_rr, murstd_r):
            rb1 = psA.tile([128, TB], F32, tag="rb1", bufs=1)
            nc.tensor.matmul(rb1, lhsT=ones_row, rhs=rstd_rr,
                             start=True, stop=True)
            rb2 = psA.tile([128, TB], F32, tag="rb2", bufs=1)
            nc.tensor.matmul(rb2, lhsT=ones_row, rhs=murstd_r,
                             start=True, stop=True)
            return rb1, rb2

        def emit_norm(tb, rb1, rb2):
            ts_ = slice(tb * TB, (tb + 1) * TB)
            for kc in range(KC):
                nc.vector.tensor_mul(xt_sb[:, kc, ts_],
                                     xt_sb[:, kc, ts_].bitcast(F32), rb1)
                nc.vector.tensor_sub(xt_sb[:, kc, ts_],
                                     xt_sb[:, kc, ts_].bitcast(F32), rb2)

        def emit_qproj(tb):
            ts_ = slice(tb * TB, (tb + 1) * TB)
            for pair in range(PAIRS):
                pq = psB.tile([128, TB], F32, tag="po", bufs=4)
                for kc in range(KC):
                    nc.tensor.matmul(pq, lhsT=wq_sb[:, kc, pair, :],
                                     rhs=xt_sb[:, kc, ts_],
                                     start=(kc == 0), stop=(kc == KC - 1))
                nc.scalar.activation(out=q_sb[:, pair, ts_], in_=pq,
                                     func=AF.Identity,
                                     bias=qb_sb[:, pair : pair + 1])

        def emit_proj(tb, include_q=True):
            ts_ = slice(tb * TB, (tb + 1) * TB)
            # kc-major across all 4 pairs (4 live PSUM accumulators): the PE
            # consumes each just-normalized kc tile for every pair at once
            # instead of head-of-line blocking on pair 0's later kc tiles
            # while the DVE normalize is still producing them.
            pks = {}
            for pair in range(PAIRS):
                pk = psB.tile([128, TB], F32, tag="po", bufs=4)
                pks[pair] = pk
            for kc in range(KC):
                for pair in range(PAIRS):
                    nc.tensor.matmul(pks[pair], lhsT=wk_sb[:, kc, pair, :],
                                     rhs=xt_sb[:, kc, ts_],
                                     start=(kc == 0), stop=(kc == KC - 1))
            for pair in range(PAIRS):
                # k bias-add on ACT (Identity w/ per-partition bias), bf16 out
                nc.scalar.activation(out=k_sb[:, pair, ts_], in_=pks[pair],
                                     func=AF.Identity,
                                     bias=kb_sb[:, pair : pair + 1])
            for tt in range(tb * 4, tb * 4 + 4):
                tts = slice(tt * 128, (tt + 1) * 128)
                pv = psB.tile([128, 512], F32, tag="po", bufs=4)
                for kc in range(KC):
                    nc.tensor.matmul(pv, lhsT=xt_sb[:, kc, tts],
                                     rhs=wv_sb[:, kc, :],
                                     start=(kc == 0), stop=(kc == KC - 1))
                nc.scalar.copy(
                    out=v_sb[:, tt, :, 0:D],
                    in_=pv.rearrange("p (h d) -> p h d", h=HPC),
                )
            if include_q:
                emit_qproj(tb)

        # interleaved emission: PE stream = stats0 rb0 stats1 proj0 rb1
        # stats2 proj1 rb2 stats3 proj2 rb3 proj3 — row chains and
        # normalizes hide under the previous tb's projections.
        s_t = {}
        r_t = {}
        s_t[0] = emit_stats(0)
        r_t[0] = emit_rows(0, *s_t[0])
        rb = emit_rb(0, *r_t[0])
        emit_norm(0, *rb)
        s_t[1] = emit_stats(1)
        r_t[1] = emit_rows(1, *s_t[1])
        # wq on the ACT queue, after tb0/tb1 Squares: arrives just before
        # proj(0)'s q projections (which run last within the block).
        nc.scalar.dma_start(out=wq_sb, in_=wq_d[:, :, :, :])
        emit_proj(0)
        for tb in range(1, NB):
            rb = emit_rb(tb, *r_t[tb])
            emit_norm(tb, *rb)
            if tb + 1 < NB:
                s_t[tb + 1] = emit_stats(tb + 1)
                r_t[tb + 1] = emit_rows(tb + 1, *s_t[tb + 1])
            # tb3's q projection is deferred into phase C's ramp-up: C only
            # needs k/v of tb3 to start, and q[tqb3] isn't read until the
            # last quarter of phase C.
            emit_proj(tb, include_q=(tb != NB - 1))

        if dbg:
            for kc in range(KC):
                nc.sync.dma_start(
                    out=z_d[kc * 128 : (kc + 1) * 128, :],
                    in_=xt_sb[:, kc, :].bitcast(F32))
            nc.sync.dma_start(out=k_dbg[:, :, :], in_=k_sb[:, :, :])
            nc.sync.dma_start(out=q_dbg[:, :, :], in_=q_sb[:, :, :])
            nc.sync.dma_start(out=v_dbg[:, :, :, :], in_=v_sb[:, :, :, :])

        rows_ctx.__exit__(None, None, None)
        sq_ctx.__exit__(None, None, None)
        psA_ctx.__exit__(None, None, None)
        wkv_ctx.__exit__(None, None, None)

        # ---------------- Phase C: attention + out-projection -----------------
        psC = ctx.enter_context(tc.tile_pool(name="psC", bufs=1, space="PSUM"))
        with tc.tile_pool(name="attn", bufs=2) as ap_, \
             tc.tile_pool(name="rows2", bufs=2) as rows2, \
             tc.tile_pool(name="drbounce", bufs=4, space="DRAM") as dram_p:
            pending_out = [None]
            pending_t = [None]
            for tqb in range(NB):
                tqs_ = slice(tqb * TB, (tqb + 1) * TB)
                obuf = ap_.tile([128, PAIRS, TB], F32R, tag="ob", bufs=2)

                def emit_tail(tpair, hh, po, tobuf):
                    # Softmax-normalize tail (one head) with NO PE work. One
                    # ACT copy evicts the whole [o|den] tile to SBUF so the
                    # PSUM bank is free for the next pair-group immediately
                    # (the DRAM-bounce round trip then happens off the po
                    # reuse path): fast-approx reciprocal of the den row,
                    # row-broadcast via DRAM-bounce DMA (gpsimd), DVE mult.
                    import concourse.bass as _b
                    drow = rows2.tile([1, TB], F32, tag="dn")
                    nc.scalar.copy(out=drow, in_=po[D : D + 1, :])
                    ot = ap_.tile([D, TB], F32, tag="ot", bufs=4)
                    nc.scalar.copy(out=ot, in_=po[0:D, :])
                    rrow = rows2.tile([1, TB], F32, tag="rr")
                    # custom-DVE ops need SBUF inputs at partition 0 on HW
                    nc.vector.reciprocal_approx_fast(out=rrow, in_=drow)
                    dr = dram_p.tile([1, TB], F32, tag="dr")
                    # write and broadcast-read on DIFFERENT queues: forces an
                    # explicit completion semaphore between them (same-queue
                    # DMA descriptors can execute on parallel channels, which
                    # intermittently raced the read against the write).
                    nc.sync.dma_start(out=dr, in_=rrow)
                    rb_ = ap_.tile([64, TB], F32, tag="rb", bufs=2)
                    bc = _b.AP(tensor=dr.tensor, offset=dr.offset,
                               ap=[[0, 64]] + [list(p) for p in dr[0, :].ap])
                    nc.gpsimd.dma_start(out=rb_, in_=bc)
                    if hh == 0:
                        nc.gpsimd.tensor_mul(tobuf[0:64, tpair, :],
                                             ot[0:D, :], rb_)
                    else:
                        tmp = ap_.tile([64, TB], F32R, tag="tmp")
                        nc.gpsimd.tensor_mul(tmp, ot[0:D, :], rb_)
                        nc.gpsimd.dma_start(out=tobuf[64:128, tpair, :],
                                            in_=tmp)

                def emit_av(vpair, vtkc, vex, vpo2):
                    # attn@V for k-tile vtkc (one pair), one step late so the
                    # PE never queues behind an in-flight exp.
                    for hh in range(2):
                        nc.tensor.matmul(
                            vpo2[hh][0 : D + 1, :],
                            lhsT=v_sb[:, vtkc, vpair * 2 + hh, :],
                            rhs=vex[:, hh * 512 : (hh + 1) * 512],
                            start=(vtkc == 0), stop=(vtkc == 2 * KC - 1))

                # Two pairs stream concurrently: pair A's exp (ACT) hides
                # under pair B's logits and attn@V, pair B's exp (DVE/
                # Schraudolph) under pair A's. Per-pair pl is [128,1024]
                # (1-tkc deep): lg(t+1) waits exp(t), which completes during
                # the other pair's 4 PE matmuls. pl 2x2 banks + po 4 = PSUM.
                for pg in range(PAIRS // 2):
                    prs = (2 * pg, 2 * pg + 1)
                    pos = {}
                    pls = {}
                    for sl, pair in enumerate(prs):
                        po0 = psB.tile([128, TB], F32, tag="po", bufs=4)
                        po1 = psB.tile([128, TB], F32, tag="po", bufs=4)
                        pos[pair] = [po0, po1]
                        pl_t = psC.tile([128, 1024], F32, tag="pl", bufs=2)
                        pls[pair] = pl_t
                    pend = []
                    for tkc in range(2 * KC):
                        pend_new = []
                        for sl, pair in enumerate(prs):
                            pl = pls[pair]
                            ex = ap_.tile([128, 1024], BF16, tag="ex", bufs=6)
                            for hh in range(2):
                                pb = hh * 64
                                nc.tensor.matmul(
                                    pl[:, hh * 512 : (hh + 1) * 512],
                                    lhsT=k_sb[pb : pb + 64, pair,
                                              tkc * 128 : (tkc + 1) * 128],
                                    rhs=q_sb[pb : pb + 64, pair, tqs_],
                                    start=True, stop=True)
                            if sl == 0:
                                nc.scalar.activation(
                                    out=ex, in_=pl[:, :],
                                    func=AF.Exp, bias=zb128[:, 0:1])
                            else:
                                # Schraudolph: bf16-exp bit pattern via one
                                # DVE tensor_scalar (f32->int16 trunc).
                                nc.vector.tensor_scalar(
                                    out=ex.bitcast(I16), in0=pl[:, :],
                                    scalar1=EXP_A, scalar2=EXP_B,
                                    op0=AluOpType.mult, op1=AluOpType.add)
                            pend_new.append((pair, tkc, ex, pos[pair]))
                        for args in pend:
                            emit_av(*args)
                        pend = pend_new
                        # spread the previous group's 4 tail chains across
                        # tkc 1..4 so the exp pipeline isn't disrupted by a
                        # burst of ACT/DVE tail work at the group boundary
                        # (carried across tqb boundaries too).
                        if pending_t[0] and 1 <= tkc <= len(pending_t[0]):
                            emit_tail(*pending_t[0][tkc - 1])
                            if tkc == len(pending_t[0]):
                                pending_t[0] = None
                        if tqb == 0 and pg == 0 and tkc == 2:
                            # deferred tb3 q-projection fills the PE while
                            # the exp pipeline is still ramping up.
                            emit_qproj(NB - 1)
                    for args in pend:
                        emit_av(*args)
                    pending_t[0] = [(pair, hh, pos[pair][hh], obuf)
                                    for pair in prs for hh in range(2)]
                    if pg == 0 and pending_out[0] is not None:
                        pending_out[0]()
                        pending_out[0] = None
                if dbg:
                    nc.sync.dma_start(out=ob_dbg[tqb, :, :, :],
                                      in_=obuf[:, :, :])

                def make_outproj(otqb, oobuf):
                    def emit():
                        for tqs in range(4):
                            osl = slice(tqs * 128, (tqs + 1) * 128)
                            osb = ap_.tile([128, 1024], F32, tag="osb", bufs=2)
                            pc0 = psB.tile([128, TB], F32, tag="po", bufs=4)
                            pc1 = psB.tile([128, TB], F32, tag="po", bufs=4)
                            for nh, pc in ((0, pc0), (1, pc1)):
                                for j in range(PAIRS):
                                    nc.tensor.matmul(
                                        pc, lhsT=oobuf[:, j, osl],
                                        rhs=wo_sb[:, j, nh * 512 : (nh + 1) * 512],
                                        start=(j == 0), stop=(j == PAIRS - 1))
                                if nh == 0:
                                    nc.scalar.copy(out=osb[:, 0:512], in_=pc)
                                else:
                                    nc.vector.tensor_copy(out=osb[:, 512:1024],
                                                          in_=pc)
                            r0 = otqb * TB + tqs * 128
                            nc.sync.dma_start(out=out_d[r0 : r0 + 128, :],
                                              in_=osb)
                    return emit

                if tqb == NB - 1:
                    for tp in pending_t[0]:
                        emit_tail(*tp)
                    pending_t[0] = None
                    make_outproj(tqb, obuf)()
                else:
                    pending_out[0] = make_outproj(tqb, obuf)
    nc.finalize()
    return nc


def get_program():
    if "nc" not in _prog_cache:
        _prog_cache["nc"] = _build_program()
    return _prog_cache["nc"]


def _round_f32r(a):
    """Round fp32 to fp32r (E8M11: 11 mantissa bits, low 12 bits zero),
    round-to-nearest-even. Matches the PE's fp32r operand precision so the
    DMA-loaded tensors satisfy walrus's 'rounded to FP32r' requirement."""
    b = np.ascontiguousarray(a, np.float32).view(np.uint32)
    lsb = (b >> np.uint32(12)) & np.uint32(1)
    r = (b + np.uint32(0x7FF) + lsb) & np.uint32(0xFFFFF000)
    return r.view(np.float32)


def _pack_inputs(x, ln_scale, ln_bias, w_qkv, w_out, b_out):
    """Returns (in_maps for 8 cores, per-batch host bias [1024])."""
    x = np.ascontiguousarray(np.asarray(x, np.float32))
    ln_scale = np.asarray(ln_scale, np.float32)
    ln_bias = np.asarray(ln_bias, np.float32)
    w_qkv = np.asarray(w_qkv, np.float32)
    w_out = np.asarray(w_out, np.float32)
    b_out = np.asarray(b_out, np.float32)

    ws = w_qkv * ln_scale[:, None]          # fold LN scale into weights
    wq_all = ws[:, 0:1024] * (D ** -0.5)    # fold 1/sqrt(d) into q
    wk_all = ws[:, 1024:2048]
    wv_all = ws[:, 2048:3072]
    qb_all = (ln_bias @ w_qkv[:, 0:1024]) * (D ** -0.5)
    kb_all = ln_bias @ w_qkv[:, 1024:2048]
    vb_all = ln_bias @ w_qkv[:, 2048:3072]
    b_eff = (b_out + vb_all @ w_out).astype(np.float32)  # host-side bias

    in_maps = []
    for core in range(N_CORES):
        b_i, g = core // 2, core % 2
        cs = slice(g * 512, (g + 1) * 512)
        # [dim, 8 heads, 64] -> pairs of heads packed along m
        wq_g = wq_all[:, cs].reshape(DIM, PAIRS, 128)   # [dim, pair, 2*64]
        wk_g = wk_all[:, cs].reshape(DIM, PAIRS, 128)
        # -> [p, kc, pair, m] so one whole-tensor DMA is contiguous
        wq_p = np.ascontiguousarray(
            wq_g.reshape(KC, 128, PAIRS, 128).transpose(1, 0, 2, 3))
        wk_p = np.ascontiguousarray(
            wk_g.reshape(KC, 128, PAIRS, 128).transpose(1, 0, 2, 3))
        wv_p = np.ascontiguousarray(
            wv_all[:, cs].reshape(KC, 128, 512).transpose(1, 0, 2))
        wo_p = np.ascontiguousarray(
            w_out[cs, :].reshape(PAIRS, 128, DIM).transpose(1, 0, 2))
        qb_p = np.ascontiguousarray(qb_all[cs].reshape(PAIRS, 128))
        kb_p = np.ascontiguousarray(kb_all[cs].reshape(PAIRS, 128))
        xt = np.ascontiguousarray(x[b_i].T)
        in_maps.append({
            "xt": _round_f32r(xt), "wq": _round_f32r(wq_p),
            "wk": _round_f32r(wk_p), "wv": _round_f32r(wv_p),
            "wo": _round_f32r(wo_p), "qb": qb_p, "kb": kb_p,
        })
    return in_maps, b_eff


def kernel(x, ln_scale, ln_bias, w_qkv, w_out, b_out):
    from concourse.bass_utils import run_bass_kernel_spmd

    nc = get_program()
    in_maps, b_eff = _pack_inputs(x, ln_scale, ln_bias, w_qkv, w_out, b_out)
    trace = bool(os.environ.get("ATTN_KERNEL_TRACE"))
    res = run_bass_kernel_spmd(nc, in_maps, core_ids=list(range(N_CORES)),
                               trace=trace)
    _prog_cache["last_exec_time_ns"] = res.exec_time_ns
    _prog_cache["last_result"] = res
    outs = res.results
    out = np.empty((B, N, DIM), np.float32)
    for b in range(B):
        out[b] = outs[2 * b]["out"] + outs[2 * b + 1]["out"] + b_eff
    return out
